# revision 17
# baseline (speedup 1.0000x reference)
"""Trainium2 Bass kernel for nn_Decoder_35837207118002 (retrieval_knn).

Problem: b=1, n_pre=8192, n_cur=16384, K=8.
  cur2pre[j] = argmin_i D[i,j]           (nearest pre for each cur)
  knn_idx[i] = 8 smallest D[i,:] (indices into cur)
  mask[i,k]  = (cur2pre[knn_idx[i,k]] == i)
  out[i]     = sum_k mask*dist / upsample[i],  dist = ||pre_i - cur_knn||

Sharding: over pre across 8 cores (1024 pre points per core), cur
replicated.  Each core computes the negated distance matrix
ND = 2*dot - psq - csq = -D against all 16384 cur points via K=5
augmented PE matmuls (host precomputes the squared norms), finds the
row top-8 per cur-quarter with the DVE max/max_index top-8 primitive
(32 candidates per pre row, a superset of the global top-8), and
reduces the column max (== -min_i D over the core's slice) via GPSIMD
running-max + PE-transpose partition reduction.

The device returns candidate values/indices and the per-core column
max; the host combines the 8 column-max slices, applies the
argmin-mask (bitwise value equality), membership threshold, exact
distance, and upsample division -- a trivially small reduction
(~256K elements of numpy) compared to the 134M-element matrix work.
"""

import numpy as np

import concourse.bass as bass
import concourse.tile as tile
import concourse.mybir as mybir
import concourse.bass_utils as bass_utils
from concourse.masks import make_identity

F32 = mybir.dt.float32
U16 = mybir.dt.uint16
AX = mybir.AxisListType
OP = mybir.AluOpType

N_CORES = 8
P = 128
N_PRE = 8192
N_CUR = 16384
K = 8
PRE_CORE = N_PRE // N_CORES      # 1024
NCH = PRE_CORE // P              # 8 pre chunks per core
NQ = 4                           # cur quarters
QW = N_CUR // NQ                 # 4096
CPR = NQ * K                     # 32 candidates per pre row
NCAND = NCH * CPR                # 256 candidate slots per partition

# This walrus build rejects 2-input TensorTensor on the Pool (gpsimd)
# engine.  The column-max merge options:
#   "dma": accumulating SBUF->SBUF DMA (accum_op=max) on the idle DMA
#          engines (SWDGE via gpsimd queue).
#   "dve": scalar_tensor_tensor on DVE reading PSUM directly.
MERGE = "dve"
# fp32r would stream matmuls at 1 cycle/row (vs 4 for plain fp32) but it
# is a reduced-precision format (inputs must be pre-rounded to fp32r) --
# the knn selection needs exact fp32, so this stays off.
F32R_MM = False

_COMPILED = {}


def _split_excess_drain_waits(nc, limit=1):
    """This walrus build encodes very few sem-waits per instruction (a
    Drain tops out at ONE).  Hoist excess waits onto preceding
    single-wait NoOps on the same engine (a NoOp doesn't stall the
    engine pipeline the way a Drain would)."""
    for f in nc.m.functions:
        for bb in f.blocks:
            insts = list(bb.instructions)
            out = []
            changed = False
            for inst in insts:
                si = inst.sync_info
                waits = list(si.on_wait) if si and si.on_wait else []
                if len(waits) > limit:
                    for kk, w in enumerate(waits[:-limit]):
                        out.append(
                            mybir.InstNoOp(
                                name=f"{inst.name}-wsplit{kk}",
                                engine=inst.engine,
                                ins=[],
                                outs=[],
                                sync_info=mybir.SyncInfo(on_wait=[w], on_update=[]),
                            )
                        )
                    si.on_wait = waits[-limit:]
                    inst.sync_info = si
                    changed = True
                out.append(inst)
            if changed:
                bb.instructions = out


def build_kernel():
    nc = bass.Bass("TRN2", target_bir_lowering=False, debug=False,
                   num_devices=N_CORES)

    pre_aug = nc.dram_tensor("pre_aug", [5, PRE_CORE], F32, kind="ExternalInput").ap()
    cur_aug = nc.dram_tensor("cur_aug", [5, N_CUR], F32, kind="ExternalInput").ap()
    oV = nc.dram_tensor("oV", [P, NCAND], F32, kind="ExternalOutput").ap()
    oI = nc.dram_tensor("oI", [P, NCAND], U16, kind="ExternalOutput").ap()
    oM = nc.dram_tensor("oM", [P, P], F32, kind="ExternalOutput").ap()

    with tile.TileContext(nc) as tc:
        with (
            tc.tile_pool(name="const", bufs=1) as const_pool,
            tc.tile_pool(name="s", bufs=3) as s_pool,
            tc.tile_pool(name="m", bufs=2) as m_pool,
            tc.tile_pool(name="mmps", bufs=3, space="PSUM") as mm_psum,
            tc.tile_pool(name="trps", bufs=2, space="PSUM") as tr_psum,
        ):
            cur_aug_sb = const_pool.tile([5, N_CUR], F32)
            nc.sync.dma_start(cur_aug_sb[:], cur_aug[:])
            pre_aug_sb = const_pool.tile([5, PRE_CORE], F32)
            nc.sync.dma_start(pre_aug_sb[:], pre_aug[:])
            ident = const_pool.tile([P, P], F32)
            make_identity(nc, ident[:])

            V_all = const_pool.tile([P, NCAND], F32)
            I_all = const_pool.tile([P, NCAND], U16)
            M_compact = const_pool.tile([P, P], F32)

            for q in range(NQ):
                Mrun = m_pool.tile([P, QW], F32, name=f"Mrun_q{q}", tag="Mrun")
                for pc in range(NCH):
                    S = s_pool.tile([P, QW], F32, name=f"S_q{q}_p{pc}", tag="S")
                    lhsT = pre_aug_sb[:, pc * P:(pc + 1) * P]
                    if F32R_MM:
                        lhsT = lhsT.bitcast(mybir.dt.float32r)
                    for t in range(QW // 1024):
                        ps = mm_psum.tile([P, 1024], F32, name=f"ps_{q}_{pc}_{t}",
                                          tag="mmps")
                        for u in range(2):
                            col = q * QW + t * 1024 + u * 512
                            rhs = cur_aug_sb[:, col:col + 512]
                            if F32R_MM:
                                rhs = rhs.bitcast(mybir.dt.float32r)
                            nc.tensor.matmul(
                                ps[:, u * 512:(u + 1) * 512],
                                lhsT,
                                rhs,
                                start=True, stop=True,
                            )
                        nc.scalar.copy(S[:, t * 1024:(t + 1) * 1024], ps[:])
                    off = pc * CPR + q * K
                    nc.vector.max(out=V_all[:, off:off + K], in_=S[:])
                    nc.vector.max_index(I_all[:, off:off + K],
                                        V_all[:, off:off + K], S[:])
                    # column-max merge: ACT copy for the first chunk, DVE
                    # elementwise max for the rest (one op per chunk).
                    if pc == 0:
                        nc.scalar.copy(Mrun[:], S[:])
                    else:
                        nc.vector.tensor_max(Mrun[:], Mrun[:], S[:])
                # partition-direction reduction of Mrun via PE transposes,
                # packed 4 per PSUM bank pair and reduced in groups.
                for tg in range(QW // P // 4):  # 8 groups of 4 tiles
                    trp = tr_psum.tile([P, 4, P], F32, name=f"trp_{q}_{tg}",
                                       tag="trps")
                    for t4 in range(4):
                        t = tg * 4 + t4
                        nc.tensor.transpose(
                            trp[:, t4], Mrun[:, t * P:(t + 1) * P], ident[:])
                    base = q * (QW // P) + tg * 4
                    nc.vector.tensor_reduce(
                        M_compact[:, base:base + 4], trp[:],
                        axis=AX.X, op=OP.max,
                    )

            nc.sync.dma_start(oV[:], V_all[:])
            nc.sync.dma_start(oI[:], I_all[:])
            nc.sync.dma_start(oM[:], M_compact[:])

    _split_excess_drain_waits(nc)
    return nc


def _prep_inputs(pre_xyzs, cur_xyzs, upsample_num):
    """Host-side per-core input prep.  Stepwise fp32 norms to mirror the
    reference's _sqdist."""
    p = np.ascontiguousarray(pre_xyzs[0], dtype=np.float32)   # (3, 8192)
    c = np.ascontiguousarray(cur_xyzs[0], dtype=np.float32)   # (3, 16384)

    psq = ((p[0] * p[0] + p[1] * p[1]) + p[2] * p[2]).astype(np.float32)
    csq = ((c[0] * c[0] + c[1] * c[1]) + c[2] * c[2]).astype(np.float32)

    cur_aug = np.empty((5, N_CUR), np.float32)
    cur_aug[0:3] = 2.0 * c
    cur_aug[3] = -1.0
    cur_aug[4] = -csq

    in_maps = []
    for core in range(N_CORES):
        s = slice(core * PRE_CORE, (core + 1) * PRE_CORE)
        pre_aug = np.empty((5, PRE_CORE), np.float32)
        pre_aug[0:3] = p[:, s]
        pre_aug[3] = psq[s]
        pre_aug[4] = 1.0
        in_maps.append({"pre_aug": pre_aug, "cur_aug": cur_aug})
    return in_maps


def kernel(pre_xyzs, cur_xyzs, upsample_num, _run_kwargs=None):
    # The bass->PJRT path needs the axon (NeuronCore) jax backend; guard
    # against a host process that pinned jax to CPU for its reference.
    try:
        import jax
        if not any("NC" in str(d) for d in jax.devices()):
            jax.config.update("jax_platforms", "axon")
    except Exception:
        pass
    if "nc" not in _COMPILED:
        _COMPILED["nc"] = build_kernel()
    nc = _COMPILED["nc"]
    in_maps = _prep_inputs(pre_xyzs, cur_xyzs, upsample_num)
    res = bass_utils.run_bass_kernel_spmd(
        nc, in_maps, core_ids=list(range(N_CORES)), **(_run_kwargs or {}))
    _COMPILED["last_results"] = res

    # ---- host-side masked reduction (tiny: ~256K elements) ----
    p = np.ascontiguousarray(pre_xyzs[0], dtype=np.float32)
    c = np.ascontiguousarray(cur_xyzs[0], dtype=np.float32)
    cur_pts = np.ascontiguousarray(c.T)                       # (16384, 3)
    up = np.ascontiguousarray(upsample_num[0], dtype=np.float32)

    m_global = np.max(
        [res.results[core]["oM"].T.reshape(-1) for core in range(N_CORES)],
        axis=0)                                               # (16384,)

    qoff = (np.arange(NQ, dtype=np.int32) * QW)[None, None, :, None]
    out = np.empty((1, N_PRE), np.float32)
    for core in range(N_CORES):
        V = res.results[core]["oV"]                           # (128, 256) f32
        I = res.results[core]["oI"].astype(np.int32)          # (128, 256)
        Ig = (I.reshape(P, NCH, NQ, K) + qoff).reshape(P, NCAND)
        Mg = m_global[Ig]
        mask = (V == Mg)
        # membership: >= 8th largest of the row's 32 candidates
        Vc = V.reshape(P, NCH, CPR)
        t8 = -np.partition(-Vc, K - 1, axis=2)[:, :, K - 1:K]
        memb = Vc >= t8
        # exact distance (reference formula) from gathered coordinates
        s = slice(core * PRE_CORE, (core + 1) * PRE_CORE)
        pre_b = np.ascontiguousarray(
            p[:, s].reshape(3, NCH, P).transpose(2, 1, 0))    # (128, 8, 3)
        CP = cur_pts[Ig].reshape(P, NCH, CPR, 3)
        diff = (CP - pre_b[:, :, None, :]).astype(np.float32)
        d2 = ((diff[..., 0] * diff[..., 0] + diff[..., 1] * diff[..., 1])
              + diff[..., 2] * diff[..., 2]).astype(np.float32)
        dist = np.sqrt(d2)
        contrib = (dist * (mask.reshape(P, NCH, CPR) & memb)).sum(
            -1, dtype=np.float32).astype(np.float32)          # (128, 8)
        ur = up[s].reshape(NCH, P)                            # (pc, p)
        out[0, s] = (contrib.T / ur).reshape(-1).astype(np.float32)
    return out
